# revision 56
# baseline (speedup 1.0000x reference)
"""FM layer (gather + segment_sum + 0.5*(s^2 - sum_sq)) on 8 Trainium2 cores.

Strategy (data parallel over batch rows):
  - core c owns batch rows [512c, 512(c+1)); batch_ids are sorted, so each
    core's nonzeros are a contiguous slice found by searchsorted.
  - host packs the embedding table as [e | e^2] f32 -> 512B rows so one
    512B gather descriptor (line-rate on SDMA) delivers both factors.
  - per core, nonzeros are sorted by (vocab bucket of 32768 rows, batch row)
    and padded per bucket to a multiple of 128 (sizes equalized across cores
    so a single SPMD program serves all 8 cores). One gpsimd.dma_gather per
    bucket (int16 indices into a 32768-row table slice).
  - batch_ids sorted => a 128-nnz tile only spans a narrow window of batch
    rows (~20-50 of 512). Per tile, the gathered rows are the matmul
    STATIONARY and a skinny window matrix is the MOVING operand:
      psum_s[0:64, w0:w1] += rows_e[128,64]^T  @ W2v [128, W]   (W2v  = onehot*v)
      psum_q[0:64, w0:w1] += rows_e2[128,64]^T @ W2v2[128, W]   (W2v2 = onehot*v^2)
    so PE moving length is W (~40) instead of 128, and s^T, q^T land on the
    same partitions 0:64 of two [64, 512] psum tiles.
  - epilogue: o^T = 0.5*(s^T*s^T - q^T) on DVE, DMA'd out as [64, 512];
    the host transposes and concatenates the 8 per-core outputs.
"""
import numpy as np

NCORES = 8
VOCAB = 1000000
EMBED = 64
BATCH = 4096
ROWS_PER_CORE = BATCH // NCORES          # 512
BUCKET = 32768
NBUCKETS = (VOCAB + BUCKET - 1) // BUCKET  # 31
D2 = 2 * EMBED                           # 128 packed row elements
WMAX = 96                                # max batch-row window per 128-nnz tile
GCAP = 1024                              # max idxs per dma_gather call


def _build_plan(feat_ids, batch_ids, feature_vals):
    """Host-side sharding plan shared by all cores.

    Returns (per_core arrays, bucket spans, tile windows). All sizes are
    equalized across cores so one SPMD program serves all 8.
    """
    feat_ids = feat_ids.astype(np.int64)
    batch_ids = batch_ids.astype(np.int64)
    core_lo = np.searchsorted(batch_ids, np.arange(NCORES) * ROWS_PER_CORE)
    core_hi = np.searchsorted(batch_ids, (np.arange(NCORES) + 1) * ROWS_PER_CORE)

    cores = []
    counts = np.zeros((NCORES, NBUCKETS), np.int64)
    for c in range(NCORES):
        lo, hi = core_lo[c], core_hi[c]
        fid = feat_ids[lo:hi]
        row = batch_ids[lo:hi] - c * ROWS_PER_CORE
        v = feature_vals[lo:hi].astype(np.float64)
        bucket = fid // BUCKET
        order = np.lexsort((row, bucket))
        fid, row, v, bucket = fid[order], row[order], v[order], bucket[order]
        cores.append((fid, row, v, bucket))
        np.add.at(counts[c], bucket, 1)

    tot_b = (np.ceil(counts.max(axis=0) / 128).astype(np.int64) * 128)  # [NBUCKETS]
    seg_off = np.concatenate([[0], np.cumsum(tot_b)])
    tot = int(seg_off[-1])
    tiles = tot // 128

    # padded per-core arrays
    idx_local = np.zeros((NCORES, tot), np.int64)
    rowarr = np.zeros((NCORES, tot), np.int64)
    valarr = np.zeros((NCORES, tot), np.float64)
    realmask = np.zeros((NCORES, tot), bool)
    for c in range(NCORES):
        fid, row, v, bucket = cores[c]
        b_start = np.searchsorted(bucket, np.arange(NBUCKETS))
        b_end = np.searchsorted(bucket, np.arange(NBUCKETS) + 1)
        for b in range(NBUCKETS):
            n = b_end[b] - b_start[b]
            o = seg_off[b]
            idx_local[c, o:o + n] = fid[b_start[b]:b_end[b]] % BUCKET
            rowarr[c, o:o + n] = row[b_start[b]:b_end[b]]
            valarr[c, o:o + n] = v[b_start[b]:b_end[b]]
            realmask[c, o:o + n] = True

    minreal = counts.min(axis=0)  # [NBUCKETS] min real nnz over cores

    # per-tile batch-row windows (min/max over cores of real rows)
    rt = rowarr.reshape(NCORES, tiles, 128)
    mt = realmask.reshape(NCORES, tiles, 128)
    big = np.where(mt, rt, 10**9)
    small = np.where(mt, rt, -1)
    w_lo = big.min(axis=(0, 2))
    w_hi = small.max(axis=(0, 2)) + 1
    empty = w_lo > w_hi - 1          # all-pad tiles
    w_lo[empty] = 0
    w_hi[empty] = 1
    widths = w_hi - w_lo
    assert widths.max() <= WMAX, f"tile window {widths.max()} > WMAX"

    return (cores, seg_off, tot_b, tot, tiles, w_lo, w_hi, idx_local, rowarr,
            valarr, realmask, minreal)


def _gather_chunks(tot_b):
    """Static per-bucket gather-call sizes (shared by program + host metadata).
    The last bucket descends so the final transfer covers few tiles."""
    sched = []
    for b in range(NBUCKETS):
        n = int(tot_b[b])
        if b == NBUCKETS - 1:
            chunks = []
            rem = n
            while rem > GCAP + 1024:
                chunks.append(GCAP)
                rem -= GCAP
            if rem > 1024 + 128:
                chunks += [rem - 1024, 512, 512]
            else:
                chunks.append(rem)
        else:
            chunks = [min(GCAP, n - g) for g in range(0, n, GCAP)]
        sched.append(chunks)
    return sched


def _build_core_arrays(c, plan):
    (cores, seg_off, tot_b, tot, tiles, w_lo, w_hi,
     idx_local, rowarr, valarr, realmask, minreal) = plan

    # side = row - w_lo of its tile (pads -> 0, their v is 0)
    tile_of = np.repeat(np.arange(tiles), 128)
    side = rowarr[c] - w_lo[tile_of]
    side[~realmask[c]] = 0
    val = valarr[c]
    val2 = 0.5 * val * val          # 0.5 epilogue factor folded in

    # idx16: per-bucket wrap-16 layout, replicated across 8 groups of 16.
    # Trailing pads get idx=-1: ucode trims trailing negatives per gather
    # call, so pads cost no HBM descriptors (their slots keep old finite
    # SBUF data; W2 rows are zero for pads, so they contribute nothing).
    idx = idx_local[c].copy()
    idx16 = np.zeros((128, tot // 16), np.int16)
    for b in range(NBUCKETS):
        o, n = int(seg_off[b]), int(tot_b[b])
        blk = idx[o:o + n].reshape(n // 16, 16).T.astype(np.int16)
        col0 = o // 16
        for g in range(8):
            idx16[g * 16:(g + 1) * 16, col0:col0 + n // 16] = blk

    sidef = side.reshape(tiles, 128).T.astype(np.float32)
    valf = val.reshape(tiles, 128).T.astype(np.float32)
    val2f = val2.reshape(tiles, 128).T.astype(np.float32)

    # per-gather-call count of real (non-pad) idxs for num_idxs_reg
    sched = _gather_chunks(tot_b)
    cnt = []
    for b in range(NBUCKETS):
        o = int(seg_off[b])
        n_real = int(realmask[c, o:o + int(tot_b[b])].sum())
        g0 = 0
        for ng in sched[b]:
            k = ng
            cnt.append(k)
            g0 += ng
    cntf = np.array([cnt], np.int32)
    return idx16, sidef, valf, val2f, cntf


def _strip_redundant_self_waits(nc):
    """Drop waits on an instruction's own engine-proc semaphore when program
    order already guarantees them (threshold <= prior same-engine updates).
    Tile emits these for pool-slot WAW; they force Bacc's one-wait-per-inst
    event-semaphore splitting, which serializes the hot loop."""
    import concourse.mybir as mybir

    proc_of_engine = {
        mybir.EngineType.PE: "PE_",
        mybir.EngineType.DVE: "DVE_",
        mybir.EngineType.Activation: "Activation_",
        mybir.EngineType.Pool: "Pool_",
        mybir.EngineType.SP: "SP_",
    }
    for blk in nc.m.functions[0].blocks:
        counts = {}
        for ins in blk.instructions:
            si = ins.sync_info
            eng = ins.engine
            pref = proc_of_engine.get(eng)
            if si is not None and si.on_wait and type(ins).__name__ == "InstDMAGatherAnt":
                # Keep the PE/Act slot-WAR leashes and the idx-load (DMAHW)
                # wait. Gathered rows are read by PE (matmul stationary) and
                # Act (q-prescale variant) only; dropping the bookkeeping
                # waits avoids Bacc's event-semaphore split serialization.
                si.on_wait = [w for w in si.on_wait
                              if (w.ant_name or "").startswith(
                                  ("PE_", "DVE_", "DMAHW", "DMASW"))]
            if si is not None and si.on_wait and pref is not None:
                kept = []
                for w in si.on_wait:
                    name = w.ant_name or ""
                    if (w.wait_mode == "sem-ge-imm"
                            and name.startswith(pref)
                            and w.wait_value <= counts.get(name, 0)):
                        continue
                    kept.append(w)
                if len(kept) != len(si.on_wait):
                    si.on_wait = kept
            if si is not None:
                for u in si.on_update:
                    name = u.ant_name or ""
                    if u.update_mode == "sem-inc" and name.startswith(
                            proc_of_engine.get(eng, "\0")):
                        counts[name] = counts.get(name, 0) + u.update_value
    return nc


def _build_bass(seg_off, tot_b, tot, tiles, w_lo, w_hi, minreal):
    import concourse.bacc as bacc
    import concourse.mybir as mybir
    from concourse.tile import TileContext

    ntl_max = int(tot_b.max()) // 128

    nc = bacc.Bacc(trn_type="TRN2")
    table = nc.dram_tensor("table", [VOCAB, D2], mybir.dt.float32, kind="ExternalInput")
    idx16 = nc.dram_tensor("idx16", [128, tot // 16], mybir.dt.int16, kind="ExternalInput")
    # meta: [ramp(WMAX) | side(tiles) | val(tiles) | val2(tiles)]
    meta = nc.dram_tensor("meta", [128, WMAX + 3 * tiles], mybir.dt.float32,
                          kind="ExternalInput")
    out = nc.dram_tensor("out", [EMBED, ROWS_PER_CORE], mybir.dt.float32,
                         kind="ExternalOutput")
    sched = _gather_chunks(tot_b)
    ncalls = sum(len(s) for s in sched)
    cnt = nc.dram_tensor("cnt", [1, ncalls], mybir.dt.int32, kind="ExternalInput")

    n0 = int(tot_b[0])  # bucket-0 idx slice loads first so gather 0 starts early

    with TileContext(nc) as tc:
        with (
            tc.tile_pool(name="const", bufs=1) as cpool,
            tc.tile_pool(name="gath", bufs=8) as gpool,
            tc.tile_pool(name="w", bufs=128) as wpool,
            tc.tile_pool(name="psum", bufs=1, space="PSUM") as ppool,
            tc.tile_pool(name="outp", bufs=1) as opool,
        ):
            idx0_sb = cpool.tile([128, n0 // 16], mybir.dt.int16, tag="idx0")
            cnt_sb = cpool.tile([1, ncalls], mybir.dt.int32, tag="cnt")
            with tc.high_priority(offset=300):
                nc.sync.dma_start(cnt_sb[:, :], cnt[:, :])
                nc.sync.dma_start(idx0_sb[:, :], idx16[:, 0:n0 // 16])
            idxr_sb = cpool.tile([128, (tot - n0) // 16], mybir.dt.int16, tag="idxr")
            nc.sync.dma_start(idxr_sb[:, :], idx16[:, n0 // 16:])
            meta_sb = cpool.tile([128, WMAX + 3 * tiles], mybir.dt.float32, tag="meta")
            nc.sync.dma_start(meta_sb[:, :], meta[:, :])
            ramp_sb = meta_sb[:, 0:WMAX]
            side_sb = meta_sb[:, WMAX:WMAX + tiles]
            val_sb = meta_sb[:, WMAX + tiles:WMAX + 2 * tiles]
            val2_sb = meta_sb[:, WMAX + 2 * tiles:WMAX + 3 * tiles]

            psum_s = ppool.tile([EMBED, ROWS_PER_CORE], mybir.dt.float32, tag="ps")
            psum_q = ppool.tile([EMBED, ROWS_PER_CORE], mybir.dt.float32, tag="pq")
            nc.vector.memset(psum_s[:, :], 0.0)
            nc.vector.memset(psum_q[:, :], 0.0)

            # last tile whose window intersects each 128-col output block;
            # the epilogue for a block runs as soon as its last writer lands.
            nblk = ROWS_PER_CORE // 128
            last_blk = [max(t for t in range(tiles)
                            if w_lo[t] < 128 * (j + 1) and w_hi[t] > 128 * j)
                        for j in range(nblk)]

            cnt_reg = nc.gpsimd.alloc_register("cnt_reg")
            t = 0
            call = 0
            ntl_chunk = (GCAP + 127) // 128
            for b in range(NBUCKETS):
                o, n = int(seg_off[b]), int(tot_b[b])
                tbl_slice = table[b * BUCKET:min((b + 1) * BUCKET, VOCAB), :]
                g0 = 0
                for ng in sched[b]:
                    ntl = ng // 128
                    # one rows tile PER gather chunk so matmuls only depend
                    # on their own chunk's transfer (per-tile dep tracking).
                    rows = gpool.tile([128, ntl_chunk, D2], mybir.dt.float32,
                                      tag="rows")
                    # zero the pad-tail region: slots skipped by the
                    # negative-idx trim must never hold NaN bit patterns
                    # (NaN*0 would poison the psum).
                    if b == 0:
                        idx_ap = idx0_sb[:, g0 // 16:(g0 + ng) // 16]
                    else:
                        oo = o - n0 + g0
                        idx_ap = idxr_sb[:, oo // 16:(oo + ng) // 16]
                    with tc.high_priority(offset=200):
                        # num_idxs_reg = per-core real count; ucode trims the
                        # trailing negative-idx pads so they cost no HBM
                        # descriptors.
                        nc.gpsimd.dma_gather(
                            rows[:, 0:ntl, :],
                            tbl_slice,
                            idx_ap,
                            ng, ng, D2, elem_step=D2,
                        )
                    g0 += ng
                    call += 1
                    for tl in range(ntl):
                        lo, hi = int(w_lo[t]), int(w_hi[t])
                        W = hi - lo
                        last = t == tiles - 1
                        wv = wpool.tile([128, WMAX], mybir.dt.float32, tag="wv")
                        nc.vector.tensor_scalar(
                            wv[:, 0:W], ramp_sb[:, 0:W],
                            side_sb[:, t:t + 1], val_sb[:, t:t + 1],
                            mybir.AluOpType.is_equal, mybir.AluOpType.mult,
                        )
                        nc.tensor.matmul(
                            psum_s[:, lo:hi], rows[:, tl, 0:EMBED], wv[:, 0:W],
                            start=False, stop=last, skip_group_check=True,
                        )
                        wv2 = wpool.tile([128, WMAX], mybir.dt.float32, tag="wv2")
                        nc.vector.tensor_scalar(
                            wv2[:, 0:W], ramp_sb[:, 0:W],
                            side_sb[:, t:t + 1], val2_sb[:, t:t + 1],
                            mybir.AluOpType.is_equal, mybir.AluOpType.mult,
                        )
                        nc.tensor.matmul(
                            psum_q[:, lo:hi], rows[:, tl, EMBED:D2], wv2[:, 0:W],
                            start=False, stop=last, skip_group_check=True,
                        )
                        # o^T = (s^T*sqrt(.5))^2 - 0.5*q^T  (0.5 folded into
                        # val2 on host), per finalized 128-col block. Act
                        # squares, DVE subtracts: no same-engine RAW chains,
                        # so the redundant-self-wait strip below stays safe.
                        for j in range(nblk):
                            if last_blk[j] != t:
                                continue
                            blk = slice(128 * j, 128 * (j + 1))
                            o1 = opool.tile([EMBED, 128], mybir.dt.float32,
                                            tag=f"o1{j}")
                            nc.scalar.activation(
                                o1[:, :], psum_s[:, blk],
                                mybir.ActivationFunctionType.Square,
                                scale=float(np.sqrt(0.5)))
                            o_sb = opool.tile([EMBED, 128], mybir.dt.float32,
                                              tag=f"o{j}")
                            nc.vector.tensor_tensor(o_sb[:, :], o1[:, :],
                                                    psum_q[:, blk],
                                                    mybir.AluOpType.subtract)
                            nc.sync.dma_start(out[:, blk], o_sb[:, :])
                        t += 1

    _strip_redundant_self_waits(nc)
    nc.compile()
    return nc


_RUN_KWARGS = {}


def kernel(feature_embedding, feature_vals, batch_ids, feat_ids, batch_size):
    from concourse.bass_utils import run_bass_kernel_spmd

    feature_embedding = np.asarray(feature_embedding, dtype=np.float32)
    feature_vals = np.asarray(feature_vals, dtype=np.float32)
    batch_ids = np.asarray(batch_ids)
    feat_ids = np.asarray(feat_ids)

    table = np.concatenate([feature_embedding, feature_embedding * feature_embedding],
                           axis=1).astype(np.float32)

    plan = _build_plan(feat_ids, batch_ids, feature_vals)
    (cores, seg_off, tot_b, tot, tiles, w_lo, w_hi,
     idx_local, rowarr, valarr, realmask, minreal) = plan

    ramp = np.broadcast_to(np.arange(WMAX, dtype=np.float32), (128, WMAX))
    in_maps = []
    for c in range(NCORES):
        idx16, sidef, valf, val2f, cntf = _build_core_arrays(c, plan)
        meta = np.concatenate([ramp, sidef, valf, val2f], axis=1).astype(np.float32)
        in_maps.append({"table": table, "idx16": idx16, "meta": meta, "cnt": cntf})

    nc = _build_bass(seg_off, tot_b, tot, tiles, w_lo, w_hi, minreal)
    res = run_bass_kernel_spmd(nc, in_maps, core_ids=list(range(NCORES)), **_RUN_KWARGS)
    out = np.concatenate(
        [res.results[c]["out"].T for c in range(NCORES)], axis=0)
    if getattr(res, "exec_time_ns", None):
        kernel.last_exec_time_ns = res.exec_time_ns
    kernel.last_results = res
    kernel.last_nc = nc
    return out.astype(np.float32)
